# revision 15
# baseline (speedup 1.0000x reference)
"""Fused LN + QKV + RoPE + attention + out-proj Trainium2 kernel.

Shapes (hardcoded from the problem spec):
  x [4, 2048, 512] fp32, w_qkv [512, 1536], w_out [512, 512],
  ln_gamma/ln_beta/b_out [512]. 8 heads of 64. Output [4, 2048, 512].

Sharding: 8 cores = 4 batches x 2 head-groups (4 heads each). Each core
computes a w_out row-split partial output for its batch; the host sums
the two partials per batch and adds b_out.

Device-side layout is feature-major (activations transposed): the QKV
weights serve directly as the matmul stationary operand, attention
scores are computed as S^T = k^T.T @ q^T so softmax's reduction lands on
the ktok partition axis where the PV matmul (with a ones-column
appended to V) performs it for free. RoPE's roll() is folded into extra
pre-rolled weight columns, ln_gamma into the weights, and ln_beta into
per-partition scalar adds fused with the RoPE combine. Matmul operands
are bf16 (fp32 PSUM accumulation); LN/softmax vector math is fp32.
"""

import numpy as np

import concourse.bass as bass
import concourse.tile as tile
from concourse import mybir
from concourse.bass_utils import run_bass_kernel_spmd

F32 = mybir.dt.float32
BF16 = mybir.dt.bfloat16
AX = mybir.AxisListType
OP = mybir.AluOpType
ACT = mybir.ActivationFunctionType

B, N, D = 4, 2048, 512
HEADS, DH = 8, 64
HPC = 4            # heads per core
EPS = 1e-5
NT = N // 128      # 16 token tiles
KT = D // 128      # 4 feature tiles


def _split_multiwait(nc):
    """Insert NoOps so no instruction carries more than one sem wait.

    The pinned walrus rejects >1 sync wait per instruction
    (setupSyncWait "Too many sync wait commands"). Waits are a
    conjunction, so hoisting all but the last onto same-engine NoOps
    immediately before the instruction is equivalent.
    """
    ctr = 0
    for fn in nc.m.functions:
        for blk in fn.blocks:
            insts = blk.instructions
            idx = 0
            while idx < len(insts):
                inst = insts[idx]
                si = inst.sync_info
                if si is not None and len(si.on_wait) > 1:
                    waits = list(si.on_wait)
                    for w in waits[:-1]:
                        nop = mybir.InstNoOp(name=f"SWNOP-{ctr}", ins=[], outs=[])
                        ctr += 1
                        nop.engine = inst.engine
                        nop.sync_info = mybir.SyncInfo(on_wait=[w], on_update=[])
                        insts.insert(idx, nop)
                        idx += 1
                    inst.sync_info = mybir.SyncInfo(
                        on_wait=[waits[-1]], on_update=list(si.on_update)
                    )
                idx += 1


def build_nc(loops=1):
    from contextlib import ExitStack

    nc = bass.Bass("TRN2", target_bir_lowering=False, num_devices=8)

    x_nat = nc.dram_tensor("x_nat", [N, D], F32, kind="ExternalInput")
    # gamma-folded QKV weights in bf16; 10 M-tiles of 128 cols:
    # q01 q23 k01 k23 v01 v23 qr01 qr23 kr01 kr23 (r = rolled for RoPE)
    wqkv = nc.dram_tensor("wqkv", [D, 1280], BF16, kind="ExternalInput")
    # per-Mtile beta contribution (beta @ W for each output col), fp32
    beta_mt = nc.dram_tensor("beta_mt", [128, 10], F32, kind="ExternalInput")
    wout = nc.dram_tensor("wout", [HPC * DH, D], BF16, kind="ExternalInput")
    cos2 = nc.dram_tensor("cos2", [128, N], F32, kind="ExternalInput")
    sin2 = nc.dram_tensor("sin2", [128, N], F32, kind="ExternalInput")
    ident = nc.dram_tensor("ident", [128, 128], BF16, kind="ExternalInput")
    y = nc.dram_tensor("y", [D, N], F32, kind="ExternalOutput")

    with tile.TileContext(nc) as tc:
      for _loop in range(loops):
        with ExitStack() as ctx:
          const = ctx.enter_context(tc.tile_pool(name="const", bufs=1))
          qk_pool = ctx.enter_context(tc.tile_pool(name="qk", bufs=1))
          vaug_pool = ctx.enter_context(tc.tile_pool(name="vaug", bufs=1))
          outn_pool = ctx.enter_context(tc.tile_pool(name="outn", bufs=1))

          ident_sb = const.tile([128, 128], BF16)
          nc.sync.dma_start(ident_sb[:], ident[:, :])
          eps_sb = const.tile([128, 1], F32)
          nc.vector.memset(eps_sb[:], EPS)
          ones_col = const.tile([1, 64], BF16)
          nc.vector.memset(ones_col[:], 1.0)
          beta_sb = const.tile([128, 10], F32)
          nc.sync.dma_start(beta_sb[:], beta_mt[:, :])
          wout_sb = []
          for kt in range(2):
              t = const.tile([128, D], BF16, tag=f"wout{kt}")
              nc.sync.dma_start(t[:], wout[kt * 128:(kt + 1) * 128, :])
              wout_sb.append(t)

          # q/k rope'd, feature-major, 2-head stacks: [128, N] bf16
          qs = [qk_pool.tile([128, N], BF16, name=f"qs{i}", tag=f"qs{i}")
                for i in range(2)]
          ks = [qk_pool.tile([128, N], BF16, name=f"ks{i}", tag=f"ks{i}")
                for i in range(2)]
          # v_aug per head-pair: 16 blocks of [v_h0|1|v_h1|1] -> [128, 2080]
          vaug = [vaug_pool.tile([128, 16 * 130], BF16, name=f"va{i}",
                                 tag=f"va{i}") for i in range(2)]
          # normalized attention output, feature-major, per head-pair
          outn = [outn_pool.tile([128, N], BF16, name=f"on{i}", tag=f"on{i}")
                  for i in range(2)]

          with ExitStack() as s1:
              ropec = s1.enter_context(tc.tile_pool(name="ropec", bufs=1))
              wq_p = s1.enter_context(tc.tile_pool(name="wq", bufs=1))
              ln_p = s1.enter_context(tc.tile_pool(name="ln", bufs=3))
              st_p = s1.enter_context(tc.tile_pool(name="st", bufs=10))
              sc_p = s1.enter_context(tc.tile_pool(name="sc", bufs=3))
              xnT_p = s1.enter_context(tc.tile_pool(name="xnT", bufs=1))
              tmp_p = s1.enter_context(tc.tile_pool(name="tmp", bufs=2))
              vsb_p = s1.enter_context(tc.tile_pool(name="vsb", bufs=1))
              pt_ps = s1.enter_context(tc.tile_pool(name="pt", bufs=2, space="PSUM"))
              qkv_ps = s1.enter_context(tc.tile_pool(name="qkvps", bufs=3, space="PSUM"))

              cos_sb = ropec.tile([128, N], F32)
              nc.sync.dma_start(cos_sb[:], cos2[:, :])
              sin_sb = ropec.tile([128, N], F32)
              nc.sync.dma_start(sin_sb[:], sin2[:, :])

              wq_sb = []
              for kt in range(KT):
                  t = wq_p.tile([128, 1280], BF16, tag=f"wq{kt}")
                  nc.sync.dma_start(t[:], wqkv[kt * 128:(kt + 1) * 128, :])
                  wq_sb.append(t)

              # ---- Stage A: LayerNorm (natural layout) + PE transpose ----
              xnT = [xnT_p.tile([128, N], BF16, name=f"xnT{ft}", tag=f"xnT{ft}")
                     for ft in range(KT)]
              for tt in range(NT):
                  xt = ln_p.tile([128, D], F32, tag="xt")
                  nc.sync.dma_start(xt[:], x_nat[tt * 128:(tt + 1) * 128, :])
                  s = st_p.tile([128, 1], F32, tag="s")
                  nc.vector.tensor_reduce(s[:], xt[:], axis=AX.X, op=OP.add)
                  sq = st_p.tile([128, 1], F32, tag="sq")
                  scr = sc_p.tile([128, D], F32, tag="scr")
                  nc.vector.scalar_tensor_tensor(
                      scr[:], xt[:], 1.0, xt[:], op0=OP.mult, op1=OP.mult,
                      accum_out=sq[:],
                  )
                  mu = st_p.tile([128, 1], F32, tag="mu")
                  nc.vector.tensor_scalar_mul(mu[:], s[:], 1.0 / D)
                  mu2 = st_p.tile([128, 1], F32, tag="mu2")
                  nc.vector.tensor_tensor(mu2[:], mu[:], mu[:], op=OP.mult)
                  var = st_p.tile([128, 1], F32, tag="var")
                  nc.vector.scalar_tensor_tensor(
                      var[:], sq[:], 1.0 / D, mu2[:], op0=OP.mult, op1=OP.subtract
                  )
                  sd = st_p.tile([128, 1], F32, tag="sd")
                  nc.scalar.activation(sd[:], var[:], ACT.Sqrt, bias=eps_sb[:])
                  rs = st_p.tile([128, 1], F32, tag="rs")
                  nc.vector.reciprocal(rs[:], sd[:])
                  xn = sc_p.tile([128, D], BF16, tag="xn")
                  nc.vector.tensor_scalar(
                      xn[:], xt[:], mu[:], rs[:], op0=OP.subtract, op1=OP.mult
                  )
                  for ft in range(KT):
                      pt = pt_ps.tile([128, 128], BF16, tag="pt")
                      nc.tensor.transpose(
                          pt[:], xn[:, ft * 128:(ft + 1) * 128], ident_sb[:]
                      )
                      nc.vector.tensor_copy(
                          xnT[ft][:, tt * 128:(tt + 1) * 128], pt[:]
                      )

              # ---- Stage B: QKV matmuls (bf16) + RoPE combine ----
              def qkv_mm(psum_ap, m, half):
                  ms = slice(m * 128, (m + 1) * 128)
                  for nn in range(2):
                      cs = slice(half * 1024 + nn * 512,
                                 half * 1024 + (nn + 1) * 512)
                      for kt in range(KT):
                          nc.tensor.matmul(
                              psum_ap[:, nn * 512:(nn + 1) * 512],
                              wq_sb[kt][:, ms], xnT[kt][:, cs],
                              start=(kt == 0), stop=(kt == KT - 1),
                          )

              def bm(m):
                  return beta_sb[:, m:m + 1]

              vsb = [vsb_p.tile([128, N], BF16, name=f"vsb{i}", tag=f"vsb{i}")
                     for i in range(2)]
              for hp in range(2):
                  for half in range(2):
                      hs = slice(half * 1024, (half + 1) * 1024)
                      for sec, dst in ((0, qs[hp]), (2, ks[hp])):
                          pq = qkv_ps.tile([128, 1024], F32, tag="pq")
                          pqr = qkv_ps.tile([128, 1024], F32, tag="pq")
                          qkv_mm(pq, sec + hp, half)
                          qkv_mm(pqr, 6 + sec + hp, half)
                          tq = tmp_p.tile([128, 1024], BF16, tag="tq")
                          # (q + beta) * cos ; (q_rolled + beta_rolled) * sin
                          nc.vector.scalar_tensor_tensor(
                              tq[:], pq[:], bm(sec + hp), cos_sb[:, hs],
                              op0=OP.add, op1=OP.mult,
                          )
                          nc.vector.scalar_tensor_tensor(
                              dst[:, hs], pqr[:], bm(6 + sec + hp), sin_sb[:, hs],
                              op0=OP.add, op1=OP.mult,
                          )
                          nc.vector.tensor_tensor(
                              dst[:, hs], dst[:, hs], tq[:], op=OP.add
                          )
                      # v (m=4+hp): add beta, keep fp32 for transpose
                      pv = qkv_ps.tile([128, 1024], F32, tag="pq")
                      qkv_mm(pv, 4 + hp, half)
                      nc.vector.tensor_scalar_add(vsb[hp][:, hs], pv[:], bm(4 + hp))

              # ---- v transpose to token-major with ones columns ----
              for hp in range(2):
                  nc.vector.memset(vaug[hp][:], 1.0)
                  va = vaug[hp].rearrange("p (m t s) -> p m t s", m=16, s=65)
                  for mt in range(NT):
                      pt = pt_ps.tile([128, 128], BF16, tag="pt")
                      nc.tensor.transpose(
                          pt[:], vsb[hp][:, mt * 128:(mt + 1) * 128], ident_sb[:]
                      )
                      nc.vector.tensor_copy(
                          va[:, mt, :, 0:64],
                          pt[:].rearrange("p (t s) -> p t s", t=2),
                      )

          # ---- Stage C: attention per head ----
          with ExitStack() as s2:
              p_pool = s2.enter_context(tc.tile_pool(name="pp", bufs=6))
              rb_p = s2.enter_context(tc.tile_pool(name="rb", bufs=2))
              oun_p = s2.enter_context(tc.tile_pool(name="oun", bufs=2))
              s_ps = s2.enter_context(tc.tile_pool(name="sps", bufs=2, space="PSUM"))
              o_ps = s2.enter_context(tc.tile_pool(name="ops", bufs=1, space="PSUM"))

              for h in range(HPC):
                  hp, hh = h // 2, h % 2
                  qof = 64 * hh
                  va = vaug[hp].rearrange("p (m t s) -> p m t s", m=16, s=65)
                  op = o_ps.tile([65, N], F32, tag="ops")
                  for mt in range(NT):
                      for h2 in range(2):
                          sp = s_ps.tile([128, 1024], F32, tag="sps")
                          for nn in range(2):
                              ns = slice(h2 * 1024 + nn * 512,
                                         h2 * 1024 + (nn + 1) * 512)
                              nc.tensor.matmul(
                                  sp[:, nn * 512:(nn + 1) * 512],
                                  ks[hp][qof:qof + 64, mt * 128:(mt + 1) * 128],
                                  qs[hp][qof:qof + 64, ns],
                                  start=True, stop=True,
                              )
                          P = p_pool.tile([128, 1024], BF16, tag="P")
                          nc.scalar.activation(P[:], sp[:], ACT.Exp,
                                               scale=float(DH) ** -0.5)
                          for nn2 in range(2):
                              ns = slice(h2 * 1024 + nn2 * 512,
                                         h2 * 1024 + (nn2 + 1) * 512)
                              nc.tensor.matmul(
                                  op[:, ns], va[:, mt, hh, :],
                                  P[:, nn2 * 512:(nn2 + 1) * 512],
                                  start=(mt == 0), stop=(mt == NT - 1),
                              )
                  # normalize: evac accumulator, reciprocal of the ones-row,
                  # PE-broadcast it across 64 partitions, multiply.
                  ounb = oun_p.tile([65, N], F32, tag="ounb")
                  nc.vector.tensor_copy(ounb[:], op[:])
                  rbr = rb_p.tile([1, N], F32, tag="rbr")
                  nc.vector.reciprocal(rbr[:], ounb[64:65, :])
                  rbr16 = rb_p.tile([1, N], BF16, tag="rbr16")
                  nc.vector.tensor_copy(rbr16[:], rbr[:])
                  rbb = o_ps.tile([64, N], F32, tag="ops")
                  for nn2 in range(4):
                      ns = slice(nn2 * 512, (nn2 + 1) * 512)
                      nc.tensor.matmul(rbb[:, ns], ones_col[:, :], rbr16[:, ns],
                                       start=True, stop=True)
                  nc.vector.tensor_tensor(
                      outn[hp][qof:qof + 64, :], ounb[0:64, :], rbb[:], op=OP.mult
                  )

          # ---- Stage D: output projection ----
          with ExitStack() as s3:
              ye_p = s3.enter_context(tc.tile_pool(name="ye", bufs=2))
              po_ps = s3.enter_context(tc.tile_pool(name="pops", bufs=2, space="PSUM"))
              for mi in range(4):
                  for half in range(2):
                      hs = slice(half * 1024, (half + 1) * 1024)
                      po = po_ps.tile([128, 1024], F32, tag="po")
                      for nn in range(2):
                          cs = slice(half * 1024 + nn * 512,
                                     half * 1024 + (nn + 1) * 512)
                          for kt in range(2):
                              nc.tensor.matmul(
                                  po[:, nn * 512:(nn + 1) * 512],
                                  wout_sb[kt][:, mi * 128:(mi + 1) * 128],
                                  outn[kt][:, cs],
                                  start=(kt == 0), stop=(kt == 1),
                              )
                      ye = ye_p.tile([128, 1024], F32, tag="ye")
                      nc.vector.tensor_copy(ye[:], po[:])
                      nc.sync.dma_start(y[mi * 128:(mi + 1) * 128, hs], ye[:])

    _split_multiwait(nc)
    return nc


def _host_prep(x, ln_gamma, ln_beta, w_qkv, w_out):
    """Build the 8 per-core input maps."""
    import ml_dtypes
    f32 = np.float32
    bf16 = ml_dtypes.bfloat16
    pos = np.arange(N, dtype=f32)[:, None]
    idx = np.arange(DH, dtype=f32)[None, :]
    angle = pos / (f32(10000.0) ** (idx / f32(DH)))       # [N, DH]
    cos2 = np.ascontiguousarray(np.tile(np.cos(angle).T, (2, 1)), dtype=f32)
    sin2 = np.ascontiguousarray(np.tile(np.sin(angle).T, (2, 1)), dtype=f32)
    ident = np.eye(128, dtype=f32).astype(bf16)

    wg = (w_qkv * ln_gamma[:, None]).astype(f32)          # [512, 1536]
    beta_row = (ln_beta @ w_qkv).astype(f32)              # [1536]

    def head_block(a, sec, h):    # sec 0=q 1=k 2=v, global head h
        return a[..., sec * 512 + h * DH: sec * 512 + (h + 1) * DH]

    in_maps = []
    for c in range(8):
        bi, hg = c // 2, c % 2
        hs = [4 * hg + i for i in range(HPC)]
        mts, bcols = [], []
        specs = [(0, 0), (0, 1), (1, 0), (1, 1), (2, 0), (2, 1)]
        for sec, p in specs:      # q01 q23 k01 k23 v01 v23
            mts.append(np.concatenate(
                [head_block(wg, sec, hs[2 * p]),
                 head_block(wg, sec, hs[2 * p + 1])], axis=1))
            bcols.append(np.concatenate(
                [head_block(beta_row, sec, hs[2 * p]),
                 head_block(beta_row, sec, hs[2 * p + 1])]))
        for sec, p in specs[:4]:  # qr01 qr23 kr01 kr23
            mts.append(np.concatenate(
                [np.roll(head_block(wg, sec, hs[2 * p]), 1, axis=1),
                 np.roll(head_block(wg, sec, hs[2 * p + 1]), 1, axis=1)], axis=1))
            bcols.append(np.concatenate(
                [np.roll(head_block(beta_row, sec, hs[2 * p]), 1),
                 np.roll(head_block(beta_row, sec, hs[2 * p + 1]), 1)]))
        wqkv_c = np.ascontiguousarray(
            np.concatenate(mts, axis=1)).astype(bf16)     # [512, 1280]
        beta_c = np.stack(bcols, axis=1).astype(f32)      # [128, 10]
        wout_c = np.ascontiguousarray(
            w_out[hg * 256:(hg + 1) * 256, :]).astype(bf16)
        in_maps.append({
            "x_nat": np.ascontiguousarray(x[bi], dtype=f32),
            "wqkv": wqkv_c,
            "beta_mt": beta_c,
            "wout": wout_c,
            "cos2": cos2,
            "sin2": sin2,
            "ident": ident,
        })
    return in_maps


_NC = None


def kernel(x, ln_gamma, ln_beta, w_qkv, w_out, b_out, **run_kwargs):
    global _NC
    x = np.asarray(x, dtype=np.float32)
    assert x.shape == (B, N, D), x.shape
    if _NC is None:
        _NC = build_nc()
    in_maps = _host_prep(np.asarray(x), np.asarray(ln_gamma),
                         np.asarray(ln_beta), np.asarray(w_qkv),
                         np.asarray(w_out))
    res = run_bass_kernel_spmd(_NC, in_maps, core_ids=list(range(8)), **run_kwargs)
    out = np.empty((B, N, D), dtype=np.float32)
    for bi in range(B):
        part = res.results[2 * bi]["y"] + res.results[2 * bi + 1]["y"]
        out[bi] = part.T + np.asarray(b_out, dtype=np.float32)
    kernel.last_results = res
    return out
